# revision 23
# baseline (speedup 1.0000x reference)
"""Causal self-attention (B=2, T=2048, C=1024, NH=16) on 8 TRN2 NeuronCores.

Sharding: core = (b, g): b in {0,1} batches, g in {0..3} head-groups of 4
heads (2 pairs of 2).  Each core computes qkv for its 4 heads from x[b],
runs causal attention, and produces a partial output projection in bf16.
The host sums the 4 partials per batch in fp32 and adds biases (bqkv's v
component is folded into bproj host-side: y += bv exactly).

v3 layout notes (per core):
  - xt (C, T) bf16: contraction dim C on partitions for qk projection.
  - q/k computed transposed: qT/kT [128 = 2 heads x 64d, pair, T] via
    W.T @ x; q weights pre-scaled by 1/8.  Bias added on DVE evict.
    wqk host layout is PAIR-major: columns [q_p0|k_p0|q_p1|k_p1] so the
    first DMA piece (cols 0:256) unblocks all of pair 0's projections.
  - v computed in NATURAL layout directly (lhsT = xt tile, rhs = Wv);
    v_sb[:, tt, h, 0:64] = values, col 64 = constant ones so the AV
    matmul also yields softmax row sums l.
  - Attention per pair pr, query-block ib (512 queries), j-tile jt (128
    keys): S^T per head [128 keys, 512 q] via row-tiled K=64 matmuls
    (tile_position (0,0)/(64,0) -> concurrent on HW) into per-head 1-bank
    PSUM tiles; exp per head on ACT; the causal mask is applied AFTER exp
    by a Pool-engine affine_select that zeroes the strict upper triangle
    of the diagonal 128x128 block of pt -- no PE or DVE cycles.  Groups
    process one off-diagonal j-tile first (covers av's full 512 width for
    the PSUM start), then the 4 DIAGONAL tiles (their exp+mask latency
    hides behind head fillers), then the remaining off-diagonals.
  - Normalize is PE-free: DVE reciprocal of the ones-column row sums
    (bf16), Pool partition_broadcast replicates 1/l down the 64 head
    partitions, DVE multiplies av -> yt.
  - Proj per token tile: two 512-column halves, each hd0+hd1 accumulated
    in a 1-bank PSUM tile and evicted separately; one out DMA per tile.
    Proj for block ib runs as fillers inside pair-1 attention group ib+1.
  - Fillers: PE work units (qk halves, v tiles, proj tiles) injected at
    fixed positions -- after diag j-tiles 2 and 4 (covers the exp+mask
    latency at group start where AV matmuls are parked blocked) and
    spread through the ACT-bound off-diagonal stretch.
  - Input DMA: weights (wqk pair pieces, wv, wproj, bias) on the gpsimd
    SWDGE queue; xt stream alone on the sync HWDGE queue, halves first.
  - PSUM (8 banks): tag "ps1" [128,512] fp32 (1 bank) x4 -- st per-head,
    qk acc, v acc, proj halves; tag "av" [65,512] fp32 (1 bank) x4 --
    AV accumulators truly double-buffered so the next group's AV stream
    never waits on the previous group's (now off-PE) normalize chain.
"""

import numpy as np

import concourse.bass as bass
import concourse.mybir as mybir
import concourse.tile as tile
from concourse import bacc
from concourse.bass import ts, ds
from concourse.bass_utils import run_bass_kernel_spmd

B, T_FULL, C = 2, 2048, 1024
NH, HD = 16, 64
N_CORES = 8
HPC = 4  # heads per core
BF16 = mybir.dt.bfloat16
FP32 = mybir.dt.float32
AF = mybir.ActivationFunctionType
ALU = mybir.AluOpType

# schedule tuning knobs (swept in the simulator)
CFG = {
    "prelude": 0,       # no-op reordering knob (kept for sweeps)
    "defer_nb": False,  # emit normalize's bcast+mul inside the next group
    "head_pos": (1, 3),  # filler positions covering the diag-head latency
    "pt_bufs": 6,
    "qkacc_tag": "av",
    "split_exp_head": 0,  # per-head exp for the first N positions of a group
    "qk_filler_split": True,  # qk fillers as two 4-matmul units
    "mask": "pool",  # 'pool': affine_select on pt; 'pe': -100 matmul fold
    "norm": "pool",  # 'pool': partition_broadcast; 'pe': selector matmul
}


def build_program(T=T_FULL, repeat=1, loop=0):
    """repeat: python-unrolled body count.  loop>0: wrap the repeats in a
    hardware For_i loop with `loop` trips (for timing: big effective
    iteration counts at small program size)."""
    NIB = T // 512   # query blocks
    NCT = C // 128   # contraction tiles
    nc = bacc.Bacc(None, target_bir_lowering=False)

    x_d = nc.dram_tensor("xt", [C, T], BF16, kind="ExternalInput")
    wqk_d = nc.dram_tensor("wqk", [C, 512], BF16, kind="ExternalInput")
    wv_d = nc.dram_tensor("wv", [C, 256], BF16, kind="ExternalInput")
    bqk_d = nc.dram_tensor("bqk", [512], FP32, kind="ExternalInput")
    wp_d = nc.dram_tensor("wproj", [256, C], BF16, kind="ExternalInput")
    out_d = nc.dram_tensor("out", [T, C], BF16, kind="ExternalOutput")

    with tile.TileContext(nc) as tc:
        with (
            tc.tile_pool(name="sb", bufs=1) as sb,
            tc.tile_pool(name="wk", bufs=1) as wk,
            tc.tile_pool(name="ps", bufs=1, space="PSUM") as ps,
        ):
            # ---- persistent SBUF (hoisted out of the repeat loop) ----
            xt_sb = sb.tile([128, NCT, T], BF16, name="xt_sb")
            wqk_sb = sb.tile([128, NCT, 512], BF16, name="wqk_sb")
            wv_sb = sb.tile([128, NCT, 256], BF16, name="wv_sb")
            wp_sb = sb.tile([128, 2, C], BF16, name="wp_sb")
            bias_sb = sb.tile([128, 4], FP32, name="bias_sb")
            qT_sb = sb.tile([128, 2, T], BF16, name="qT_sb")
            kT_sb = sb.tile([128, 2, T], BF16, name="kT_sb")
            v_sb = sb.tile([128, T // 128, HPC, 65], BF16, name="v_sb")
            yt_sb = sb.tile([128, 2, T], BF16, name="yt_sb")

            nc.vector.memset(v_sb[:, :, :, 64:65], 1.0)

            if CFG["mask"] == "pe":
                # identity (PE mask-add lhsT) and the causal -100 triangle
                id_sb = sb.tile([128, 128], BF16, name="id_sb")
                mask_sb = sb.tile([128, 2, 128], BF16, name="mask_sb")
                nc.vector.memset(id_sb[:, :], 1.0)
                nc.gpsimd.affine_select(
                    out=id_sb[:, :], in_=id_sb[:, :],
                    compare_op=ALU.is_equal, fill=0.0, base=0,
                    pattern=[[1, 128]], channel_multiplier=-1,
                )
                nc.vector.memset(mask_sb[:, :, :], 0.0)
                nc.gpsimd.affine_select(
                    out=mask_sb[:, :, :], in_=mask_sb[:, :, :],
                    compare_op=ALU.is_ge, fill=-100.0, base=0,
                    pattern=[[0, 2], [1, 128]], channel_multiplier=-1,
                )
            if CFG["norm"] == "pe":
                sel_sb = sb.tile([64, 128], BF16, name="sel_sb")
                rl_sb = sb.tile([64, 2, T], BF16, name="rl_sb")
                nc.vector.memset(sel_sb[:, :], 0.0)
                nc.vector.memset(sel_sb[0:1, 0:64], 1.0)
                nc.vector.memset(sel_sb[32:33, 64:128], 1.0)
                nc.vector.memset(rl_sb[0:64, :, :], 0.0)

            def body():
                # ---- input DMA ----
                # Weights ride the gpsimd SWDGE queue (parallel to HWDGE);
                # the xt stream has the sync HWDGE queue to itself.  wqk is
                # DMA'd in pair-column pieces so pair 0's projections can
                # start after the first piece.
                def dma_wqk(pr):
                    nc.gpsimd.dma_start(
                        out=wqk_sb[:, :, ds(256 * pr, 256)],
                        in_=wqk_d[:, ds(256 * pr, 256)].rearrange(
                            "(c p) f -> p c f", p=128
                        ),
                    )
                def dma_xt(ci0, nci, tp):
                    nc.sync.dma_start(
                        out=xt_sb[:, ds(ci0, nci), ts(tp, 512)],
                        in_=x_d[ds(128 * ci0, 128 * nci), ts(tp, 512)].rearrange(
                            "(c p) f -> p c f", p=128
                        ),
                    )
                dma_wqk(0)
                dma_xt(0, 4, 0)
                nc.gpsimd.dma_start(
                    out=bias_sb[:, :],
                    in_=bqk_d.ap().rearrange("(a p) -> p a", p=128),
                )
                nc.gpsimd.dma_start(
                    out=wv_sb[:, :, :],
                    in_=wv_d.ap().rearrange("(c p) f -> p c f", p=128),
                )
                dma_xt(4, 4, 0)
                dma_wqk(1)
                nc.gpsimd.dma_start(
                    out=wp_sb[:, :, :],
                    in_=wp_d.ap().rearrange("(a p) f -> p a f", p=128),
                )
                for tp in range(1, NIB):
                    dma_xt(0, 8, tp)

                # ---- building blocks ----
                def qk_half(pr, tp, which):
                    """q or k projection for pair pr, 512-token chunk tp."""
                    cb = 2 * pr + which  # column block in pair-major wqk
                    dest = qT_sb if which == 0 else kT_sb
                    # st-tagged (not "av") so head fillers never contend with
                    # the av accumulators still held by the previous group's
                    # in-flight normalize chain.
                    if CFG["qkacc_tag"] == "st":
                        acc = ps.tile([128, 2, 512], FP32, name="qkacc",
                                      tag="st", bufs=2)[:, 0, :]
                    else:
                        acc = ps.tile([128, 512], FP32, name="qkacc",
                                      tag="av", bufs=4)
                    for ci in range(NCT):
                        nc.tensor.matmul(
                            acc[:, :],
                            wqk_sb[:, ci, ts(cb, 128)],
                            xt_sb[:, ci, ts(tp, 512)],
                            start=(ci == 0),
                            stop=(ci == NCT - 1),
                        )
                    nc.vector.tensor_scalar_add(
                        dest[:, pr, ts(tp, 512)],
                        acc[:, :],
                        bias_sb[:, cb : cb + 1],
                    )

                def v_tile(tt):
                    """v in natural layout for token tile tt (4 heads)."""
                    vacc = ps.tile([128, 256], FP32, name="vacc",
                                   tag="av", bufs=4)
                    for ci in range(NCT):
                        nc.tensor.matmul(
                            vacc[:, :],
                            xt_sb[:, ci, ts(tt, 128)],
                            wv_sb[:, ci, :],
                            start=(ci == 0),
                            stop=(ci == NCT - 1),
                        )
                    nc.vector.tensor_copy(
                        v_sb[:, tt, :, 0:64],
                        vacc.rearrange("p (h d) -> p h d", h=HPC),
                    )

                def group_order(pr, ib):
                    njt = 4 * (ib + 1)
                    diag = list(range(4 * ib, njt))
                    offd = list(range(0, 4 * ib))
                    if offd:
                        return [offd[0]] + diag + offd[1:]
                    return diag

                def produce_tile(pr, ib, jt, split_exp=False):
                    """QK^T -> exp -> (Pool causal mask if diagonal) for one
                    128-key j-tile; returns the pt handle for the AV step.
                    split_exp: per-head exp instrs (lower latency to the
                    first AV; used at group heads where ACT has slack)."""
                    a = jt - 4 * ib
                    is_diag = a >= 0
                    off = 128 * a if is_diag else 0
                    w = 512 - off
                    st = ps.tile([128, 2, 512], FP32, name="st",
                                 tag="st", bufs=2)
                    pt = wk.tile([128, 2, 512], BF16, name="pt",
                                 tag="pt", bufs=CFG["pt_bufs"])
                    pe_mask = is_diag and CFG["mask"] == "pe"
                    for h2 in range(2):
                        nc.tensor.matmul(
                            st[:, h2, ds(off, w)],
                            kT_sb[ds(64 * h2, 64), pr, ts(jt, 128)],
                            qT_sb[ds(64 * h2, 64), pr,
                                  ds(512 * ib + off, w)],
                            start=True,
                            stop=not pe_mask,
                            skip_group_check=pe_mask,
                        )
                    if pe_mask:
                        nc.tensor.matmul(
                            st[:, :, ds(off, 128)],
                            id_sb[:, :],
                            mask_sb[:, :, :],
                            start=False,
                            stop=True,
                            skip_group_check=True,
                        )
                    if split_exp:
                        for h2 in range(2):
                            nc.scalar.activation(
                                pt[:, h2, ds(off, w)],
                                st[:, h2, ds(off, w)],
                                AF.Exp,
                            )
                    else:
                        nc.scalar.activation(
                            pt[:, :, ds(off, w)], st[:, :, ds(off, w)], AF.Exp
                        )
                    if is_diag and CFG["mask"] == "pool":
                        # causal mask: zero the strict upper triangle of
                        # the diagonal 128x128 block (query q < key p)
                        nc.gpsimd.affine_select(
                            out=pt[:, :, ds(off, 128)],
                            in_=pt[:, :, ds(off, 128)],
                            compare_op=ALU.is_ge, fill=0.0, base=0,
                            pattern=[[0, 2], [1, 128]],
                            channel_multiplier=-1,
                        )
                    return (jt, off, w, pt)

                def attn_group(pr, ib, fillers=(), prelude=(), norm_b=None):
                    """AV accumulation over the group's j-tiles.  `prelude`:
                    tiles already produced at the previous group's tail.
                    `norm_b` (prev group's broadcast+mul) is emitted after
                    the diagonal tiles so Pool masks aren't head-of-line
                    blocked behind it."""
                    njt = 4 * (ib + 1)
                    order = group_order(pr, ib)
                    head_pos = CFG["head_pos"] if 4 * ib else (1, 3)
                    od_start = (head_pos[-1] + 1) if fillers else 5
                    nb_pos = min(4 if 4 * ib else 3, njt - 1)
                    fillers = list(fillers)
                    n_od = max(0, len(fillers) - len(head_pos))
                    n_od_tiles = max(0, njt - od_start)
                    od_gap = max(2, n_od_tiles // n_od) if n_od else 1
                    av = [
                        ps.tile([65, 512], FP32, name=f"av{h2}",
                                tag="av", bufs=4)
                        for h2 in range(2)
                    ]
                    prelude = list(prelude)
                    for pos, jt in enumerate(order):
                        if pos < len(prelude):
                            prod = prelude[pos]
                            assert prod[0] == jt
                        else:
                            prod = produce_tile(
                                pr, ib, jt,
                                split_exp=pos < CFG["split_exp_head"],
                            )
                        _, off, w, pt = prod
                        for h2 in range(2):
                            nc.tensor.matmul(
                                av[h2][:, ds(off, w)],
                                v_sb[:, jt, 2 * pr + h2, :],
                                pt[:, h2, ds(off, w)],
                                start=(pos == 0),
                                stop=(pos == njt - 1),
                            )
                        if fillers and (
                            pos in head_pos
                            or (pos >= od_start
                                and (pos - od_start) % od_gap == od_gap - 1
                                and pos < njt - 1)
                        ):
                            fillers.pop(0)()
                        if norm_b is not None and pos == nb_pos:
                            norm_b()
                            norm_b = None
                    for f in fillers:
                        f()
                    if norm_b is not None:
                        norm_b()
                    return av

                def normalize_a(pr, ib, av):
                    """reciprocal of the ones-column row sums (DVE only)."""
                    blk = ts(ib, 512)
                    # bf16 1/l: 0.4% multiplicative error, inside tolerance
                    if CFG["norm"] == "pe":
                        with nc.allow_low_precision(reason="bf16 1/l bcast"):
                            for h2 in range(2):
                                nc.vector.reciprocal(
                                    rl_sb[32 * h2 : 32 * h2 + 1, pr, blk],
                                    av[h2][64:65, :],
                                )
                        return None
                    # per-head [64,...] tiles: the partition_broadcast ucode
                    # reads the source on Q7 core 0 (absolute partitions
                    # 0-15) and writes absolute partitions [0, channels), so
                    # both src and dst must be partition-0 based.
                    rbc = [
                        wk.tile([64, 512], BF16, name=f"rbc{h2}",
                                tag="rbcsb", bufs=4)
                        for h2 in range(2)
                    ]
                    with nc.allow_low_precision(reason="bf16 1/l broadcast"):
                        for h2 in range(2):
                            nc.vector.reciprocal(
                                rbc[h2][0:1, :], av[h2][64:65, :]
                            )
                    return rbc

                def normalize_b(pr, ib, av, rbc):
                    """broadcast of 1/l down head partitions + DVE mul -> yt."""
                    blk = ts(ib, 512)
                    if CFG["norm"] == "pe":
                        rbcp = ps.tile([128, 2, 512], FP32, name="rbcp",
                                       tag="st", bufs=2)
                        nc.tensor.matmul(
                            rbcp[:, 0, :],
                            sel_sb[0:33, :],
                            rl_sb[0:33, pr, blk],
                            start=True,
                            stop=True,
                        )
                        rbc = wk.tile([128, 512], BF16, name="rbc",
                                      tag="rbcsb", bufs=2)
                        nc.vector.tensor_copy(rbc[:, :], rbcp[:, 0, :])
                        for h2 in range(2):
                            nc.vector.tensor_mul(
                                yt_sb[ds(64 * h2, 64), pr, blk],
                                av[h2][0:64, :],
                                rbc[ds(64 * h2, 64), :],
                            )
                        return
                    for h2 in range(2):
                        nc.gpsimd.partition_broadcast(
                            rbc[h2][0:64, :],
                            rbc[h2][0:1, :],
                            channels=64,
                        )
                    for h2 in range(2):
                        nc.vector.tensor_mul(
                            yt_sb[ds(64 * h2, 64), pr, blk],
                            av[h2][0:64, :],
                            rbc[h2][0:64, :],
                        )

                def proj_tile(tt, last=False):
                    """output projection + DMA for one token tile; two
                    512-col halves, each a 1-bank PSUM accumulation."""
                    outst = wk.tile([128, 1024], BF16, name="outst",
                                    tag="outst", bufs=2)
                    for oc in range(2):
                        pp = ps.tile([128, 512], FP32, name="pp",
                                     tag="av", bufs=4)
                        for hd in range(2):
                            nc.tensor.matmul(
                                pp[:, :],
                                yt_sb[:, hd, ts(tt, 128)],
                                wp_sb[:, hd, ds(512 * oc, 512)],
                                start=(hd == 0),
                                stop=(hd == 1),
                            )
                        # final block: split evictions DVE/ACT (shorter tail)
                        if last and (tt * 2 + oc) % 2 == 1:
                            nc.scalar.activation(
                                outst[:, ds(512 * oc, 512)], pp[:, :], AF.Copy
                            )
                        else:
                            nc.vector.tensor_copy(
                                outst[:, ds(512 * oc, 512)], pp[:, :]
                            )
                    nc.sync.dma_start(out=out_d[ts(tt, 128), :], in_=outst[:, :])

                # ---- schedule ----
                def qk_half_units(pr, tp, which):
                    """qk_half as filler unit(s); optionally split into two
                    4-matmul pieces sharing one accumulation."""
                    if not CFG["qk_filler_split"]:
                        return [lambda: qk_half(pr, tp, which)]
                    state = {}

                    def piece(lo, hi, first, last):
                        def run():
                            if first:
                                if CFG["qkacc_tag"] == "st":
                                    state["acc"] = ps.tile(
                                        [128, 2, 512], FP32, name="qkacc",
                                        tag="st", bufs=2)[:, 0, :]
                                else:
                                    state["acc"] = ps.tile(
                                        [128, 512], FP32, name="qkacc",
                                        tag="av", bufs=4)
                            acc = state["acc"]
                            cb = 2 * pr + which
                            for ci in range(lo, hi):
                                nc.tensor.matmul(
                                    acc[:, :],
                                    wqk_sb[:, ci, ts(cb, 128)],
                                    xt_sb[:, ci, ts(tp, 512)],
                                    start=(ci == 0),
                                    stop=(ci == NCT - 1),
                                )
                            if last:
                                dest = qT_sb if which == 0 else kT_sb
                                nc.vector.tensor_scalar_add(
                                    dest[:, pr, ts(tp, 512)],
                                    acc[:, :],
                                    bias_sb[:, cb : cb + 1],
                                )
                        return run
                    h = NCT // 2
                    return [piece(0, h, True, False),
                            piece(h, NCT, False, True)]

                qk_chunk_fillers = lambda pr, tp: (
                    qk_half_units(pr, tp, 0) + qk_half_units(pr, tp, 1)
                )
                qk_half(0, 0, 0)
                qk_half(0, 0, 1)
                for tt in range(4):
                    v_tile(tt)
                groups = [(0, ib) for ib in range(NIB)] + [
                    (1, ib) for ib in range(NIB)
                ]
                pending_nb = None  # previous group's deferred normalize_b
                prelude = ()
                for gi, (pr, ib) in enumerate(groups):
                    fillers = []
                    if pr == 0:
                        nxt = (0, ib + 1) if ib + 1 < NIB else (1, 0)
                        fillers += qk_chunk_fillers(*nxt)
                        if ib + 1 < NIB:
                            fillers += [
                                (lambda t=tt: v_tile(t))
                                for tt in range(4 * (ib + 1), 4 * (ib + 1) + 4)
                            ]
                    else:
                        if ib + 1 < NIB:
                            fillers += qk_chunk_fillers(1, ib + 1)
                        if ib > 0:
                            fillers += [
                                (lambda t=tt: proj_tile(t))
                                for tt in range(4 * (ib - 1), 4 * (ib - 1) + 4)
                            ]
                    av = attn_group(pr, ib, fillers, prelude,
                                    norm_b=pending_nb if CFG["defer_nb"]
                                    else None)
                    if pending_nb is not None and not CFG["defer_nb"]:
                        pass  # already emitted inline below
                    if gi + 1 < len(groups):
                        npr, nib = groups[gi + 1]
                        norder = group_order(npr, nib)
                        prelude = [
                            produce_tile(npr, nib, jt)
                            for jt in norder[: CFG["prelude"]]
                        ]
                    else:
                        prelude = ()
                    rbc = normalize_a(pr, ib, av)
                    nb = (lambda p=pr, i=ib, a=av, r=rbc:
                          normalize_b(p, i, a, r))
                    if CFG["defer_nb"] and gi + 1 < len(groups):
                        pending_nb = nb
                    else:
                        nb()
                        pending_nb = None
                proj_last = NIB - 1
                for tt in range(4 * proj_last, 4 * proj_last + 4):
                    proj_tile(tt, last=True)

            if loop > 0:
                with tc.For_i(0, loop, 1):
                    for _ in range(repeat):
                        body()
            else:
                for _ in range(repeat):
                    body()

    nc.compile()
    return nc


def _prep_inputs(x, Wqkv, bqkv, Wproj, T=T_FULL):
    """Build the 8 per-core input maps (host-side shard/cast/transpose)."""
    import ml_dtypes

    bf16 = ml_dtypes.bfloat16
    x = np.asarray(x, dtype=np.float32)
    Wqkv = np.asarray(Wqkv, dtype=np.float32)
    bqkv = np.asarray(bqkv, dtype=np.float32)
    Wproj = np.asarray(Wproj, dtype=np.float32)

    in_maps = []
    for b in range(B):
        xt = np.ascontiguousarray(x[b, :T].T).astype(bf16)  # (C, T)
        for g in range(N_CORES // B):
            heads = [4 * g + h for h in range(HPC)]
            # pair-major column layout: [q_p0 | k_p0 | q_p1 | k_p1]
            wqk_cols = []
            bqk_cols = []
            for pr in range(2):
                ph = heads[2 * pr : 2 * pr + 2]
                wq = np.concatenate(
                    [Wqkv[:, hh * HD : (hh + 1) * HD] for hh in ph], axis=1
                ) * 0.125
                wk_ = np.concatenate(
                    [Wqkv[:, C + hh * HD : C + (hh + 1) * HD] for hh in ph],
                    axis=1,
                )
                wqk_cols += [wq, wk_]
                bq = np.concatenate(
                    [bqkv[hh * HD : (hh + 1) * HD] for hh in ph]
                ) * 0.125
                bk = np.concatenate(
                    [bqkv[C + hh * HD : C + (hh + 1) * HD] for hh in ph]
                )
                bqk_cols += [bq, bk]
            wqk = np.concatenate(wqk_cols, axis=1).astype(bf16)  # (C, 512)
            bqk = np.concatenate(bqk_cols).astype(np.float32)  # (512,)
            wv = np.concatenate(
                [Wqkv[:, 2 * C + hh * HD : 2 * C + (hh + 1) * HD] for hh in heads],
                axis=1,
            ).astype(bf16)  # (C, 256)
            wp = np.concatenate(
                [Wproj[hh * HD : (hh + 1) * HD, :] for hh in heads], axis=0
            ).astype(bf16)  # (256, C)
            in_maps.append({"xt": xt, "wqk": wqk, "wv": wv, "bqk": bqk, "wproj": wp})
    return in_maps


_PROGRAM_CACHE = {}


def get_program(T=T_FULL, repeat=1, loop=0):
    key = (T, repeat, loop)
    if key not in _PROGRAM_CACHE:
        _PROGRAM_CACHE[key] = build_program(T, repeat, loop)
    return _PROGRAM_CACHE[key]


def kernel(x, Wqkv, bqkv, Wproj, bproj):
    x = np.asarray(x)
    in_dtype = x.dtype
    nc = get_program(T_FULL)
    in_maps = _prep_inputs(x, Wqkv, bqkv, Wproj)
    res = run_bass_kernel_spmd(nc, in_maps, list(range(N_CORES))).results
    gpb = N_CORES // B
    bqkv = np.asarray(bqkv, dtype=np.float32)
    bproj = np.asarray(bproj, dtype=np.float32)
    # fold the v bias exactly: y = attn(x) + bv  =>  out += bv @ Wproj
    bproj_eff = bproj + np.asarray(bqkv[2 * C :], dtype=np.float32) @ np.asarray(
        Wproj, dtype=np.float32
    )
    out = np.stack(
        [
            sum(res[b * gpb + g]["out"].astype(np.float32) for g in range(gpb))
            + bproj_eff
            for b in range(B)
        ]
    )
    return out.astype(in_dtype)
